# revision 100
# baseline (speedup 1.0000x reference)
"""Trainium2 Bass kernel for nn_AttEncoder (per-channel Conv1d encoder + tiny
cross-channel attention + residual).

Reference computation (B=4, C=4, L=32000, F3=1536, K=16, stride=8):
  feat[b,c,:,t] = Conv1d(x[b,c], W[c])        -> split into k,q,v  [B,C,N,T], N=512
  w[b,i,j,t]    = sum_f k[b,j,f,t] q[b,i,f,t]
  w             = softmax over j
  out           = (w @ v + v) * 0.5           -> [B,C,N,T], T=3999

Algebraic restructuring used here: q,k,v are linear in the 16-tap input
windows X_c[k,t] = x[c, 8t+k], so
  w[i,j,t]  = sum_{k,k'} M_ij[k,k'] X_i[k,t] X_j[k',t],  M_ij = Wq_i^T Wk_j  (16x16)
  out[i,f,t] = sum_{j,k} 0.5*Wv[j,f,k] * w'[i,j,t] X_j[k,t],  w' = softmax(w)+I
This avoids materializing the 3*N feature maps entirely; per 500-column chunk
the whole pipeline is 7 matmuls + a handful of DVE/ACT ops.

Sharding: (batch b, T-half h) across 8 cores; attention is pointwise in t and
the conv is local, so there are no collectives. Halves overlap at t=1999.
"""

import numpy as np
from contextlib import ExitStack

import concourse.bass as bass
import concourse.tile as tile
from concourse import bacc, mybir
from concourse.bass_utils import run_bass_kernel_spmd

# problem constants (hardcoded per the self-contained contract)
B, C, L = 4, 4, 32000
F3, KW, STRIDE = 1536, 16, 8
NF = F3 // 3                     # 512 features per q/k/v
T = (L - KW) // STRIDE + 1       # 3999
TC = 2000                        # t-columns per core
CH = 500                         # chunk of t per inner iteration
NCH = TC // CH                   # 4
LC = STRIDE * (TC - 1) + KW      # 16008 input samples per core
T0 = (0, 1999)                   # per-half starting t (halves overlap at 1999)

F32 = mybir.dt.float32
F32R = mybir.dt.float32r


def _r(ap):
    # reinterpret an fp32 AP as float32r: same bits, 4x faster PE matmul at
    # reduced multiply precision (well inside this problem's tolerance)
    return ap.bitcast(mybir.dt.float32r)


def _pairpos(i, j):
    # row position of channel-pair (i,j) in the score layout. Compute-engine
    # APs may only start at partitions 0/32/64/96, so the four j-groups live
    # at quadrant offsets: rows {32q+i} share i and cover all j (enables the
    # partition-tree sum over j with legal offsets), and the diagonal pairs
    # (i==j, q=0) occupy rows 0..3 (enables the single +1 residual add).
    return 32 * ((j - i) % 4) + i


def _build_consts(W):
    """CPU-side weight preprocessing. W: [C, F3, 1, KW] float32."""
    Wd = W.astype(np.float64)
    Wk = Wd[:, 0:NF, 0, :]           # [4, 512, 16]
    Wq = Wd[:, NF:2 * NF, 0, :]
    Wv = Wd[:, 2 * NF:3 * NF, 0, :]
    # M[i,j,k,k'] = sum_f Wq[i,f,k] * Wk[j,f,k']
    M = np.einsum("ifk,jfl->ijkl", Wq, Wk)

    # Row layout of the 128-row working tiles, per i-pair tile ip (i in
    # {2ip, 2ip+1}): row r = g*64 + jp*16 + k holds X_{jp}[k,t] with g the
    # replica index (xk_rep = [Xstack; Xstack]).
    # Column layout c = i_rel*64 + j*16 + k'.
    wm = np.zeros((2, 128, 128), np.float32)   # block placement of M
    wr = np.zeros((2, 128, 100), np.float32)   # k'-sum -> quadrant score rows
    wb = np.zeros((2, 100, 128), np.float32)   # score row -> 128-row broadcast
    wd = np.zeros((2, 4, 128), np.float32)     # +se[i] on diagonal rows
    wbr = np.zeros((2, 4, 128), np.float32)    # 1/se[i] -> 128-row broadcast
    for ip in range(2):
        for ir in range(2):
            ia = 2 * ip + ir
            for j in range(4):
                r0 = ir * 64 + ia * 16       # rows (g=ir, jp=ia, k)
                c0 = ir * 64 + j * 16        # cols (i_rel=ir, j, k')
                wm[ip, r0:r0 + 16, c0:c0 + 16] = M[ia, j]
                wr[ip, c0:c0 + 16, _pairpos(ia, j)] = 1.0
                wb[ip, _pairpos(ia, j), c0:c0 + 16] = 1.0
                wbr[ip, ia, c0:c0 + 16] = 1.0
                if j == ia:
                    wd[ip, ia, c0:c0 + 16] = 1.0
    # ones-pattern summing the 4 quadrant rows sharing i -> sum over j
    ls = np.zeros((100, 4), np.float32)
    for q in range(4):
        for i in range(4):
            ls[32 * q + i, i] = 1.0
    # wv[(j,k), f] = 0.5 * Wv[j, f, k]  (the 0.5 output scale folded in)
    wv = np.zeros((64, NF), np.float32)
    for j in range(4):
        wv[j * 16:(j + 1) * 16, :] = 0.5 * Wv[j].T
    return wm, wr, wb, wd, wbr, ls, wv


def _emit(ctx, tc, o, xs, wm, wr, wb, wd, wbr, ls, wv, ident):
    nc = tc.nc
    consts = ctx.enter_context(tc.tile_pool(name="consts", bufs=1))
    xin = ctx.enter_context(tc.tile_pool(name="xin", bufs=3))
    xnp = ctx.enter_context(tc.tile_pool(name="xn", bufs=16))
    upool = ctx.enter_context(tc.tile_pool(name="u", bufs=6))
    small = ctx.enter_context(tc.tile_pool(name="small", bufs=3))
    oc = ctx.enter_context(tc.tile_pool(name="oc", bufs=24))
    ppool = ctx.enter_context(tc.tile_pool(name="pp", bufs=2, space="PSUM"))
    xtp = ctx.enter_context(tc.tile_pool(name="xt", bufs=2, space="PSUM"))
    wspool = ctx.enter_context(tc.tile_pool(name="wsp", bufs=1, space="PSUM"))
    sepool = ctx.enter_context(tc.tile_pool(name="sep", bufs=1, space="PSUM"))
    avpool = ctx.enter_context(tc.tile_pool(name="av", bufs=2, space="PSUM"))

    wm_s = consts.tile([128, 256], F32R)
    wr_s = consts.tile([128, 200], F32R)
    wb_s = consts.tile([100, 256], F32R)
    wd_s = consts.tile([4, 256], F32R)
    wbr_s = consts.tile([4, 256], F32R)
    ls_s = consts.tile([100, 4], F32R)
    # wv duplicated into rows 0-63 and 64-127 so the lhsT slice for either
    # half of uv matches the rhs base partition (matmul requires equality)
    wv_s = consts.tile([128, NF], F32R)
    id_s = consts.tile([128, 128], F32)
    # identity on the ACT queue so it lands in parallel with chunk-0's
    # window loads on the SP queue
    nc.scalar.dma_start(id_s[:], ident[:, :])

    def _load_consts():
        # split across the ACT and SP HWDGE queues (both mostly idle at
        # startup), ordered by when the pipeline first needs each weight so
        # none of them gates a chunk-0 stage
        loads = []
        for ip in range(2):
            loads += [(wm_s[:, ip * 128:(ip + 1) * 128], wm[ip]),
                      (wr_s[:, ip * 100:(ip + 1) * 100], wr[ip])]
        loads.append((ls_s[:], ls[:, :]))
        for ip in range(2):
            loads += [(wb_s[:, ip * 128:(ip + 1) * 128], wb[ip]),
                      (wd_s[:, ip * 128:(ip + 1) * 128], wd[ip]),
                      (wbr_s[:, ip * 128:(ip + 1) * 128], wbr[ip])]
        for ip in range(2):
            loads.append((wv_s[ip * 64:(ip + 1) * 64, :], wv[:, :]))
        for idx, (dst, srcap) in enumerate(loads):
            eng = nc.scalar if idx % 2 == 0 else nc.sync
            eng.dma_start(dst, srcap)

    TB = 125  # t-block for the transpose stage (4 blocks per chunk)
    ncopy = 0
    av_gate = None   # set to a chunk-0 PE gate; late transposes order after
    chunks = [(i * 500, 500) for i in range(NCH)]
    for t_off, CH in chunks:
        # xk_rep [128, CH]: row (g,j,k) = x[j, 8*(t_off + t) + k].
        # DMA inner dims must be contiguous, so the strided window gather is
        # loaded in natural [t, (j,k)] layout and transposed on the PE; both
        # 64-row replica halves are copied from the same transposed tile.
        xk = xin.tile([128, CH], F32)
        xt0 = xtp.tile([64, CH], F32, tag="xt")   # PSUM
        for blk in range(CH // TB):
            xn = xnp.tile([TB, 64], F32)   # [t, (j,k)]
            src = bass.AP(xs.tensor, STRIDE * (t_off + TB * blk),
                          [[STRIDE, TB], [LC, 4], [1, KW]])
            # window loads split across the SP HWDGE queue and Pool SWDGE
            # (~1.16us/load serial) so every chunk's xn lands before the PE
            # reaches its in-order transpose slot — a transpose stalling on
            # its load blocks chunk-0's score chain behind it
            if t_off == 0:
                nc.sync.dma_start(xn[:], src)
            else:
                nc.gpsimd.dma_start(xn[:], src)
            cs = slice(blk * TB, (blk + 1) * TB)
            tmm = nc.tensor.matmul(xt0[:, cs], xn[:],
                                   id_s[0:TB, 0:TB], start=True, stop=True)
            gate = av_gate
            if t_off >= 1000 and gate is not None:
                # keep late chunks' transposes out of the PE stream until
                # chunk 0's output chain has issued (they otherwise stall
                # the in-order PE on their trickling SWDGE loads)
                tile.add_dep_helper(tmm.ins, gate, sync=False,
                                    reason="defer late transposes")
        if t_off == 0:
            _load_consts()
        # both replica halves read the same transposed tile
        for g in range(2):
            nc.vector.tensor_copy(_r(xk[g * 64:(g + 1) * 64, :]), xt0[:])

        # scores: P = blockdiag(M)^T @ xk_rep ; U = P .* xk_rep ;
        # ws[16, CH] = sum_{k'} U  (accumulated over both i-pair tiles)
        us = []
        for ip in range(2):
            p = ppool.tile([128, CH], F32, tag="pp")
            nc.tensor.matmul(p[:], wm_s[:, ip * 128:(ip + 1) * 128],
                             _r(xk[:]), start=True, stop=True)
            u = upool.tile([128, CH], F32, tag="u")
            nc.vector.tensor_mul(_r(u[:]), p[:], xk[:])
            us.append(u)
        ws = wspool.tile([100, CH], F32)
        nc.tensor.matmul(ws[:], wr_s[:, 0:100], _r(us[0][:]),
                         start=True, stop=False)
        nc.tensor.matmul(ws[:], wr_s[:, 100:200], _r(us[1][:]),
                         start=False, stop=True)

        # softmax over j, done with PE ones-pattern matmuls: sum over the 4
        # quadrant rows sharing i, reciprocal on DVE, then broadcast both the
        # un-normalized exp'd scores (+ se[i] on diagonal rows for the
        # residual identity) and 1/se back to the 128-row layout.
        ew = small.tile([100, CH], F32, tag="ew")
        nc.scalar.activation(_r(ew[:]), ws[:],
                             mybir.ActivationFunctionType.Exp)
        sep = sepool.tile([4, CH], F32)
        nc.tensor.matmul(sep[:], ls_s[:], _r(ew[:]), start=True, stop=True)
        rc = small.tile([4, CH], F32, tag="rc")
        nc.vector.reciprocal(_r(rc[:]), sep[:])
        ses = small.tile([4, CH], F32, tag="ses")
        nc.scalar.copy(_r(ses[:]), sep[:])

        # output: w' = (ew + I*se)/se broadcast to 128 rows, weight xk_rep,
        # contract with Wv
        for ip in range(2):
            wrep = ppool.tile([128, CH], F32, tag="pp")
            nc.tensor.matmul(wrep[:], wb_s[:, ip * 128:(ip + 1) * 128],
                             _r(ew[:]), start=True, stop=False)
            nc.tensor.matmul(wrep[:], wd_s[:, ip * 128:(ip + 1) * 128],
                             _r(ses[:]), start=False, stop=True)
            rrep = ppool.tile([128, CH], F32, tag="pp")
            rrmm = nc.tensor.matmul(rrep[:], wbr_s[:, ip * 128:(ip + 1) * 128],
                                    _r(rc[:]), start=True, stop=True)
            if t_off == 0 and ip == 0:
                av_gate2 = rrmm.ins
            tmp = upool.tile([128, CH], F32, tag="u")
            nc.vector.tensor_mul(tmp[:], wrep[:], xk[:])
            uv = upool.tile([128, CH], F32, tag="u")
            nc.vector.tensor_mul(_r(uv[:]), tmp[:], rrep[:])
            for ir in range(2):
                ia = 2 * ip + ir

                for fb in range(4):
                    av = avpool.tile([128, CH], F32)
                    avmm = nc.tensor.matmul(av[:],
                                            wv_s[ir * 64:(ir + 1) * 64,
                                                 fb * 128:(fb + 1) * 128],
                                            _r(uv[ir * 64:(ir + 1) * 64, :]),
                                            start=True, stop=True)
                    # DMA can't read PSUM; bounce through SBUF, splitting
                    # copies between DVE and ACT by engine headroom
                    ob = oc.tile([128, CH], F32, tag="ob")
                    if ncopy % 2 < 1:
                        nc.vector.tensor_copy(ob[:], av[:])
                    else:
                        nc.scalar.copy(ob[:], av[:])
                    if t_off == 0 and ip == 0 and fb == 0:
                        av_gate = av_gate2
                    ncopy += 1
                    # one DMA per f-block, launched right after its copy
                    # (HWDGE has headroom at this DMA size)
                    nc.sync.dma_start(
                        o[ia, fb * 128:(fb + 1) * 128, t_off:t_off + CH],
                        ob[:])


def _build_nc():
    nc = bacc.Bacc("TRN2", target_bir_lowering=False, debug=False,
                   num_devices=8)
    xs = nc.dram_tensor("xs", [C, LC], F32, kind="ExternalInput").ap()
    wm = nc.dram_tensor("wm", [2, 128, 128], F32R, kind="ExternalInput").ap()
    wr = nc.dram_tensor("wr", [2, 128, 100], F32R, kind="ExternalInput").ap()
    wb = nc.dram_tensor("wb", [2, 100, 128], F32R, kind="ExternalInput").ap()
    wd = nc.dram_tensor("wd", [2, 4, 128], F32R, kind="ExternalInput").ap()
    wbr = nc.dram_tensor("wbr", [2, 4, 128], F32R, kind="ExternalInput").ap()
    ls = nc.dram_tensor("ls", [100, 4], F32R, kind="ExternalInput").ap()
    wv = nc.dram_tensor("wv", [64, NF], F32R, kind="ExternalInput").ap()
    ident = nc.dram_tensor("ident", [128, 128], F32, kind="ExternalInput").ap()
    o = nc.dram_tensor("o", [C, NF, TC], F32, kind="ExternalOutput").ap()
    with tile.TileContext(nc) as tc, ExitStack() as ctx, \
            nc.allow_low_precision(reason="float32r is 32-bit storage; "
                                   "rounding is well inside tolerance"):
        _emit(ctx, tc, o, xs, wm, wr, wb, wd, wbr, ls, wv, ident)
    nc.compile()
    return nc


_NC_CACHE = None


def _make_in_maps(x, W):
    wm, wr, wb, wd, wbr, ls, wv = _build_consts(W)
    ident = np.eye(128, dtype=np.float32)
    in_maps = []
    for core in range(8):
        b, h = core // 2, core % 2
        xs = np.ascontiguousarray(
            x[b, :, STRIDE * T0[h]: STRIDE * T0[h] + LC], dtype=np.float32)
        in_maps.append({"xs": xs, "wm": wm, "wr": wr, "wb": wb, "wd": wd,
                        "wbr": wbr, "ls": ls, "wv": wv, "ident": ident})
    return in_maps


def kernel(x, W, _trace=False, _trace_kwargs=None):
    global _NC_CACHE
    if _NC_CACHE is None:
        _NC_CACHE = _build_nc()
    nc = _NC_CACHE
    in_maps = _make_in_maps(np.asarray(x), np.asarray(W))
    kw = {}
    if _trace:
        kw = dict(trace=True, **(_trace_kwargs or {}))
    try:
        res = run_bass_kernel_spmd(nc, in_maps, core_ids=list(range(8)), **kw)
    except Exception:
        # transient device wedges (e.g. NRT_EXEC_UNIT_UNRECOVERABLE) clear
        # on re-dispatch; retry once before giving up
        res = run_bass_kernel_spmd(nc, in_maps, core_ids=list(range(8)), **kw)
    out = np.empty((B, C, NF, T), np.float32)
    for core in range(8):
        b, h = core // 2, core % 2
        oarr = res.results[core]["o"]
        if h == 0:
            out[b, :, :, 0:TC] = oarr
        else:
            out[b, :, :, T0[1] + 1:T] = oarr[:, :, 1:]
    if _trace:
        return out, res
    return out
